# revision 20
# baseline (speedup 1.0000x reference)
"""Trainium2 Bass kernel for nn_HHGR (gnn_message_passing).

Strategy (8 NeuronCores, groups sharded 1024/core):
  host prep: sum_tab = user_table + user_embedding; two member slabs per
  core: memb (natural [128, NT*128] bf16 rows = member rows, used as the
  g_att stationary) and membt (transposed [D, R] fp8e4 scaled x16, used
  as the moving operand of the hidden-layer matmul -> no PE transposes);
  H^T slab bf16 with k-rows permuted to the AllGather-half layout;
  gtab^T; mask block-diag in (slice, chunk) column order so batched
  logits land contiguously; replicated attention weights.

  device per core (1024 groups = 8 superblocks x 128 groups):
  * hid^T = aw1^T @ membt per 512-row chunk, 3 chunks stacked per PSUM
    bank (partition offsets 0/32/64); batched Relu+bias per stack;
    logits batched: lhsT = stacked hidT 128-col slice, rhs = aw2sel
    [128, 3] -> 512/384 member rows per matmul (96 matmuls/core vs 256).
  * softmax: one Exp+bias; pm = p*mask; dent = ones^T @ pm (direct
    transposed denominators, no PE transpose); denbc via 4 K=1 ind4
    matmuls; maskp = pm * recip.
  * g_att^T accumulated per 128-row tile (lhsT = memb tile, rhs = maskp
    4 cols); X^T = g_att^T + gtab^T; V = X @ hw1 (natural rows).
  * software pipelining: exp/pm of sb issue before hid/logits of sb+1,
    then the softmax-dependent PE work of sb — the DVE chain hides
    under the next superblock's matmuls.
  * V AllGathered in 2 halves (after superblocks 3/7); W in 2 halves
    produced by column-half-major stage 1. All collective triggers and
    slab-prefetch DMA triggers live on the GpSimd queue; vd stores on
    the vector queue right after the V copy.
  * stage 1 (Y^T = V^T H^T, K=8192) column-half-major, k-ordered so the
    second V half is consumed last (ch0) / first (ch1); N=512 matmuls.
  * stage 2 (out^T = W^T H^T) k-ordered by W half arrival; out^T
    [128, 1024] f32 written once and transposed on the host.
"""
import sys
sys.path.insert(0, "/opt/trn_rl_repo")

import numpy as np
import ml_dtypes

import concourse.bass as bass  # noqa: F401
import concourse.bacc as bacc
import concourse.mybir as mybir
import concourse.tile as tile
from concourse.bass_utils import run_bass_kernel_spmd

F32 = mybir.dt.float32
BF16 = mybir.dt.bfloat16
FP8 = mybir.dt.float8e4
AF = mybir.ActivationFunctionType

G, M, D, U = 8192, 32, 128, 200000
H_ATT = 16
NC = 8
GPC = G // NC          # 1024 groups per core
R = GPC * M            # 32768 member rows per core
NT = R // 128          # 256 tiles of 128 rows
NSB = GPC // 128       # 8 superblocks of 128 groups (32 tiles each)
KT = G // 128          # 64 k-tiles for the big matmuls
MT_SCALE = 16.0        # membt fp8 pre-scale (folded into aw1)

_CACHE = {}


def _build():
    nc = bacc.Bacc("TRN2", target_bir_lowering=False, debug=False)

    memb = nc.dram_tensor("memb", [128, NT * D], FP8, kind="ExternalInput")
    membt = nc.dram_tensor("membt", [128, NT * D], FP8, kind="ExternalInput")
    mdiag = nc.dram_tensor("mdiag", [128, NT * 4], BF16, kind="ExternalInput")
    gtabt = nc.dram_tensor("gtabt", [D, GPC], F32, kind="ExternalInput")
    ht = nc.dram_tensor("ht", [G, GPC], BF16, kind="ExternalInput")
    aw1 = nc.dram_tensor("aw1", [D, H_ATT], BF16, kind="ExternalInput")
    aw2sel = nc.dram_tensor("aw2sel", [128, 3], BF16, kind="ExternalInput")
    ab1s = nc.dram_tensor("ab1s", [128, 1], F32, kind="ExternalInput")
    ab2s = nc.dram_tensor("ab2s", [128, 1], F32, kind="ExternalInput")
    hw1 = nc.dram_tensor("hw1", [D, D], BF16, kind="ExternalInput")
    hw2 = nc.dram_tensor("hw2", [D, D], BF16, kind="ExternalInput")
    ind4 = nc.dram_tensor("ind4", [1, 512], F32, kind="ExternalInput")
    ones_col = nc.dram_tensor("ones_col", [128, 1], BF16, kind="ExternalInput")

    outt = nc.dram_tensor("outt", [D, GPC], F32, kind="ExternalOutput")

    # collective staging: per-half V/W rows, tiled [128, 4*128] so the
    # post-AllGather reload is 1KB-contiguous per partition.
    vd = nc.dram_tensor("vd", [128, GPC], BF16)
    vag = nc.dram_tensor("vag", [NC * 128, GPC], BF16, addr_space="Shared")
    wd = [nc.dram_tensor(f"wd{h}", [128, 512], BF16) for h in range(2)]
    wag = [
        nc.dram_tensor(f"wag{h}", [NC * 128, 512], BF16, addr_space="Shared")
        for h in range(2)
    ]

    def allgather(src, dst):
        nc.gpsimd.collective_compute(
            "AllGather",
            mybir.AluOpType.bypass,
            replica_groups=[list(range(NC))],
            ins=[src[:]],
            outs=[dst[:]],
        )

    with tile.TileContext(nc) as tc:
        with (
            tc.tile_pool(name="const", bufs=1) as cpool,
            tc.tile_pool(name="htpool", bufs=1) as htpool,
            tc.tile_pool(name="gath", bufs=2) as gpool,
            tc.tile_pool(name="gatht", bufs=2) as tpool,
            tc.tile_pool(name="kagg", bufs=1) as kpool,
            tc.tile_pool(name="work", bufs=2) as wpool,
            tc.tile_pool(name="big", bufs=1) as xpool,
            tc.tile_pool(name="ps_h", bufs=2, space="PSUM") as psh,
            tc.tile_pool(name="ps_m", bufs=2, space="PSUM") as psm,
            tc.tile_pool(name="ps_w", bufs=2, space="PSUM") as psw,
            tc.tile_pool(name="ps_y", bufs=1, space="PSUM") as psy,
        ):
            # ---- constants ----
            aw1_sb = cpool.tile([D, H_ATT], BF16, tag="aw1")
            nc.sync.dma_start(aw1_sb[:], aw1[:])
            aw2sel_sb = cpool.tile([128, 3], BF16, tag="aw2sel")
            nc.sync.dma_start(aw2sel_sb[:], aw2sel[:])
            ab1s_sb = cpool.tile([128, 1], F32, tag="ab1s")
            nc.sync.dma_start(ab1s_sb[:], ab1s[:])
            ab2s_sb = cpool.tile([128, 1], F32, tag="ab2s")
            nc.sync.dma_start(ab2s_sb[:], ab2s[:])
            hw1_sb = cpool.tile([D, D], BF16, tag="hw1")
            nc.sync.dma_start(hw1_sb[:], hw1[:])
            hw2_sb = cpool.tile([D, D], BF16, tag="hw2")
            nc.sync.dma_start(hw2_sb[:], hw2[:])
            ind4_sb = cpool.tile([1, 512], F32, tag="ind4")
            nc.sync.dma_start(ind4_sb[:], ind4[:])
            ones_sb = cpool.tile([128, 1], BF16, tag="ones")
            nc.sync.dma_start(ones_sb[:], ones_col[:])
            mdiag_sb = cpool.tile([128, NT * 4], BF16, tag="mdiag")
            nc.sync.dma_start(mdiag_sb[:], mdiag[:])
            gtabt_sb = cpool.tile([D, GPC], F32, tag="gtabt")
            nc.sync.dma_start(gtabt_sb[:], gtabt[:])

            # engine warmups: first ACT/DVE ops pay a ucode library
            # load; do it on tiny data at t0 instead of mid-pipeline
            wu_sb = cpool.tile([128, 2], F32, tag="wusb")
            nc.scalar.activation(wu_sb[:, 0:1], ab2s_sb[:], AF.Exp)
            nc.vector.reciprocal(wu_sb[:, 1:2], wu_sb[:, 0:1])

            # ---- H^T resident [128, KT*GPC] bf16 (16MB) ----
            # NOT loaded up-front: the 64 k-tile loads are issued from the
            # scalar queue inside the attention loop (paced by the exp data
            # dependency), so the member slabs get full HBM bandwidth first
            # and the attention runs PE-bound instead of DMA-starved.
            ht_sb = htpool.tile([128, KT * GPC], BF16, tag="ht")
            vstage = xpool.tile([128, GPC], BF16, tag="vstage")

            def load_ht(k):
                nc.gpsimd.dma_start(
                    ht_sb[:, k * GPC : (k + 1) * GPC],
                    ht[k * 128 : (k + 1) * 128, :],
                )

            # ---- member slab prefetch (gpsimd trigger queue) ----
            def load_gbig(sb):
                t = gpool.tile([128, 32 * 128], FP8, tag="gbig", name=f"gbig{sb}")
                nc.gpsimd.dma_start(
                    t[:], memb[:, sb * 32 * 128 : (sb + 1) * 32 * 128]
                )
                return t

            def load_gbt(sb):
                t = tpool.tile([128, 32 * 128], FP8, tag="gbt", name=f"gbt{sb}")
                nc.gpsimd.dma_start(
                    t[:], membt[:, sb * 32 * 128 : (sb + 1) * 32 * 128]
                )
                return t

            gbt = [None] * NSB
            gbig = [None] * NSB
            gbt[0] = load_gbt(0)
            gbig[0] = load_gbig(0)
            if NSB > 1:
                gbt[1] = load_gbt(1)
                gbig[1] = load_gbig(1)

            miscs = [None] * NSB

            def attn_A(sb):
                """hid + logits for superblock sb -> logit area of misc."""
                misc = psm.tile([128, 512], F32, tag="misc", name=f"misc{sb}")
                miscs[sb] = misc
                logit_ps = misc[:, 0:32]
                hid_list = []
                # stacks: q=0 chunks 0-2, q=1 chunks 3-5, q=2 chunks 6-7
                for q in range(3):
                    nch = 3 if q < 2 else 2
                    hid_ps = psh.tile([128, 512], F32, tag="hid",
                                      name=f"hid{sb}_{q}")
                    for j in range(nch):
                        c = 3 * q + j
                        nc.tensor.matmul(
                            hid_ps[32 * j : 32 * j + H_ATT, :],
                            aw1_sb[:],
                            gbt[sb][:, c * 512 : (c + 1) * 512],
                            start=True,
                            stop=True,
                        )
                    hid_list.append((hid_ps, nch))
                # relu (ACT) + batched logits; interleave so ACT of stack q
                # overlaps PE of stack q+1's hid above / logits below
                for q, (hid_ps, nch) in enumerate(hid_list):
                    hidT = wpool.tile([128, 512], BF16, tag="hidT",
                                      name=f"hidT{sb}_{q}")
                    nc.scalar.activation(
                        hidT[:], hid_ps[:], AF.Relu, bias=ab1s_sb[:, :1]
                    )
                    for s in range(4):
                        # logit col = s*8 + c for chunk c = 3q+j
                        nc.tensor.matmul(
                            logit_ps[:, s * 8 + 3 * q : s * 8 + 3 * q + nch],
                            hidT[:, s * 128 : (s + 1) * 128],
                            aw2sel_sb[:, 0:nch],
                            start=True,
                            stop=True,
                        )

            def attn_exp(sb):
                """early ACT/DVE for B(sb): exp + mask-multiply."""
                misc = miscs[sb]
                p_sb = wpool.tile([128, 32], BF16, tag="p", name=f"p{sb}")
                nc.scalar.activation(
                    p_sb[:], misc[:, 0:32], AF.Exp, bias=ab2s_sb[:, :1]
                )
                pm_sb = wpool.tile([128, 128], BF16, tag="pm", name=f"pm{sb}")
                nc.vector.tensor_tensor(
                    pm_sb[:].rearrange("p (t l) -> p t l", l=4),
                    p_sb[:].rearrange("p (t o) -> p t o", o=1).to_broadcast(
                        [128, 32, 4]
                    ),
                    mdiag_sb[:, sb * 128 : (sb + 1) * 128].rearrange(
                        "p (t l) -> p t l", l=4
                    ),
                    mybir.AluOpType.mult,
                )
                return pm_sb

            def attn_B(sb, pm_sb):
                """softmax denominators + g_att + V for superblock sb."""
                misc = miscs[sb]
                dent_ps = misc[0:1, 32:160]
                denbc_ps = misc[:, 160:192]
                gatt_ps = misc[:, 192:320]
                v_ps = misc[:, 320:448]

                # transposed denominators: dent[0, col] = sum_rows pm[row, col]
                nc.tensor.matmul(dent_ps, ones_sb[:], pm_sb[:],
                                 start=True, stop=True)
                dent_sb = wpool.tile([1, 128], F32, tag="dent", name=f"dent{sb}")
                nc.vector.tensor_copy(dent_sb[:], dent_ps)
                # den_bc[r, j] = den[(j, r//32)] via 4 K=1 matmuls
                dent_re = dent_sb[:].rearrange("p (t l) -> p t l", l=4)
                for gl in range(4):
                    nc.tensor.matmul(
                        denbc_ps,
                        ind4_sb[0:1, gl * 128 : (gl + 1) * 128],
                        dent_re[:, :, gl : gl + 1],
                        start=(gl == 0),
                        stop=(gl == 3),
                    )
                recip_sb = wpool.tile([128, 32], F32, tag="recip",
                                      name=f"recip{sb}")
                nc.vector.reciprocal(recip_sb[:], denbc_ps)
                maskp_sb = wpool.tile([128, 128], BF16, tag="maskp",
                                      name=f"maskp{sb}")
                nc.vector.tensor_tensor(
                    maskp_sb[:].rearrange("p (t l) -> p t l", l=4),
                    recip_sb[:].rearrange("p (t o) -> p t o", o=1).to_broadcast(
                        [128, 32, 4]
                    ),
                    pm_sb[:].rearrange("p (t l) -> p t l", l=4),
                    mybir.AluOpType.mult,
                )
                # g_att^T accumulation: [128 d, 128 groups]; tile t's
                # weights live at logit col j = (t%4)*8 + t//4
                for t in range(32):
                    jt = (t % 4) * 8 + t // 4
                    nc.tensor.matmul(
                        gatt_ps[:, 4 * t : 4 * t + 4],
                        gbig[sb][:, t * 128 : (t + 1) * 128],
                        maskp_sb[:, 4 * jt : 4 * jt + 4],
                        start=True,
                        stop=True,
                    )
                xt_sb = wpool.tile([128, 128], BF16, tag="xt", name=f"xt{sb}")
                nc.vector.tensor_tensor(
                    xt_sb[:],
                    gatt_ps,
                    gtabt_sb[:, sb * 128 : (sb + 1) * 128],
                    mybir.AluOpType.add,
                )
                nc.tensor.matmul(v_ps, xt_sb[:], hw1_sb[:], start=True, stop=True)
                nc.vector.tensor_copy(
                    vstage[:, sb * 128 : (sb + 1) * 128], v_ps
                )

            # ---- attention main loop (software pipelined) ----
            ht_per_sb = [0, 2, 4, 6, 8, 9, 10, 0]
            ht_next = 0
            attn_A(0)
            for sb in range(NSB):
                if sb + 2 < NSB:
                    gbt[sb + 2] = load_gbt(sb + 2)
                    gbig[sb + 2] = load_gbig(sb + 2)
                for _ in range(ht_per_sb[sb]):
                    load_ht(ht_next)
                    ht_next += 1
                pm_sb = attn_exp(sb)
                if sb + 1 < NSB:
                    attn_A(sb + 1)
                attn_B(sb, pm_sb)
                if sb == NSB - 1:
                    nc.sync.dma_start(vd[:], vstage[:])
                    allgather(vd, vag)
            # ht tail: issued after the AG doorbell so the gpsimd queue
            # reaches the doorbell with no slot barriers ahead of it
            while ht_next < KT:
                load_ht(ht_next)
                ht_next += 1
            vag_sb = kpool.tile([128, NC * GPC], BF16, tag="kqv")
            for q in range(4):
                c0, c1 = 2 * q, 2 * q + 2
                nc.sync.dma_start(
                    vag_sb[:, c0 * GPC : c1 * GPC].rearrange(
                        "p (c f) -> p c f", f=GPC
                    ),
                    vag[c0 * 128 : c1 * 128, :].rearrange(
                        "(c p) f -> p c f", p=128
                    ),
                )

            # ---- stage 1: Y^T = V^T H^T ----
            # blocks: (vh0,ch0), (vh0,ch1), (vh1,ch0) -> y0 done at the 3/4
            # point -> W half 0 + AllGather launch early, then (vh1,ch1).
            ht_all = xpool.tile([128, GPC], BF16, tag="hT")
            wag_sb = [None, None]
            y_tiles = [
                psy.tile([128, 512], F32, tag="y0", name="y_ps0"),
                psy.tile([128, 512], F32, tag="y1", name="y_ps1"),
            ]

            def finish_w_half(ch):
                nc.scalar.activation(
                    ht_all[:, ch * 512 : (ch + 1) * 512], y_tiles[ch][:], AF.Relu
                )
                w_half = wpool.tile([128, 512], BF16, tag="w",
                                    name=f"w{ch}")
                for gb in range(4):
                    g0 = ch * 4 + gb
                    w_ps = psw.tile([128, 128], F32, tag="wps",
                                    name=f"wps{ch}_{gb}")
                    nc.tensor.matmul(
                        w_ps[:],
                        ht_all[:, g0 * 128 : (g0 + 1) * 128],
                        hw2_sb[:],
                        start=True,
                        stop=True,
                    )
                    nc.vector.tensor_copy(
                        w_half[:, gb * 128 : (gb + 1) * 128], w_ps[:]
                    )
                nc.sync.dma_start(wd[ch][:], w_half[:])
                allgather(wd[ch], wag[ch])

            for ch in range(2):
                for kt in range(KT):
                    nc.tensor.matmul(
                        y_tiles[ch][:],
                        vag_sb[:, kt * 128 : (kt + 1) * 128],
                        ht_sb[:, kt * GPC + ch * 512 : kt * GPC + ch * 512 + 512],
                        start=(kt == 0),
                        stop=(kt == KT - 1),
                    )
                finish_w_half(ch)
            for ch in range(2):
                wag_sb[ch] = kpool.tile(
                    [128, NC * 512], BF16, tag=f"kqw{ch}", name=f"wag_sb{ch}"
                )
                nc.sync.dma_start(
                    wag_sb[ch][:].rearrange("p (c f) -> p c f", f=512),
                    wag[ch].rearrange("(c p) f -> p c f", p=128),
                )

            # ---- stage 2: out^T = W^T H^T, k-ordered by W half ----
            o_ps = [
                psy.tile([128, 512], F32, tag="y0", name="o_ps0"),
                psy.tile([128, 512], F32, tag="y1", name="o_ps1"),
            ]
            for c2 in range(2):
                first = True
                for wh in range(2):
                    for c in range(NC):
                        for gb in range(4):
                            kidx = c * 8 + wh * 4 + gb
                            last = wh == 1 and c == NC - 1 and gb == 3
                            nc.tensor.matmul(
                                o_ps[c2][:],
                                wag_sb[wh][:, (c * 4 + gb) * 128 : (c * 4 + gb + 1) * 128],
                                ht_sb[:, kidx * GPC + c2 * 512 : kidx * GPC + c2 * 512 + 512],
                                start=first,
                                stop=last,
                            )
                            first = False
                # copy/store of this half overlaps the next pass's matmuls
                ot_sb = xpool.tile([128, 512], F32, tag=f"ot{c2}")
                nc.vector.tensor_copy(ot_sb[:], o_ps[c2][:])
                nc.sync.dma_start(outt[:, c2 * 512 : (c2 + 1) * 512], ot_sb[:])

    nc.compile()
    return nc


def _prep_inputs(group_inputs, members, member_mask, user_embedding, H_gl,
                 user_table, group_table, aw1, ab1, aw2, ab2, hw1, hw2):
    bf = ml_dtypes.bfloat16
    f8 = ml_dtypes.float8_e4m3
    sum_tab = (
        np.asarray(user_table, np.float32) + np.asarray(user_embedding, np.float32)
    )
    gi = np.asarray(group_inputs, np.int64)
    gtab_full = np.asarray(group_table, np.float32)[gi]
    Hg = np.asarray(H_gl, np.float32)

    aw2v = np.asarray(aw2, np.float32).reshape(-1)
    ab1v = np.asarray(ab1, np.float32).reshape(-1)
    aw2sel = np.zeros((128, 3), np.float32)
    ab1s = np.zeros((128, 1), np.float32)
    for j in range(4):
        ab1s[32 * j : 32 * j + H_ATT, 0] = ab1v
        if j < 3:
            aw2sel[32 * j : 32 * j + H_ATT, j] = aw2v
    ab2s = np.full((128, 1), np.asarray(ab2, np.float32).reshape(-1)[0], np.float32)
    ind4 = np.zeros((1, 512), np.float32)
    for gl in range(4):
        ind4[0, gl * 128 + 32 * gl : gl * 128 + 32 * (gl + 1)] = 1.0

    # single-V-AllGather layout: k-tile kt = c*8 + j holds global groups
    # c*GPC + j*128 + p, i.e. the identity ordering
    perm = np.arange(G)

    consts = dict(
        aw1=(np.asarray(aw1, np.float32) / MT_SCALE).astype(bf),
        aw2sel=aw2sel.astype(bf),
        ab1s=ab1s,
        ab2s=ab2s,
        hw1=(np.asarray(hw1, np.float32) / MT_SCALE).astype(bf),
        hw2=np.asarray(hw2, np.float32).astype(bf),
        ind4=ind4,
        ones_col=np.ones((128, 1), np.float32).astype(bf),
    )

    p = np.arange(128)
    gl_p = p // 32
    m_p = p % 32
    # logit col j covers superblock-local tile t(j) = 4*(j%8) + j//8
    t_of_j = 4 * (np.arange(32) % 8) + np.arange(32) // 8
    cols = np.arange(NSB * 32)                       # (sb, j) column index
    grp_of_col = (cols // 32) * 128 + 4 * t_of_j[cols % 32]  # local group base
    in_maps = []
    for c in range(NC):
        sl = slice(c * GPC, (c + 1) * GPC)
        mem = np.asarray(members, np.int64)[sl].astype(np.int32).reshape(-1)
        mask01 = (np.asarray(member_mask, np.float32)[sl] > 0).astype(np.float32)
        # mdiag[p, (sb, j, l)] = mask of (group base+l, member p%32) if p//32==l
        val = mask01[(grp_of_col[None, :] + gl_p[:, None]), m_p[:, None]]
        mdiag = np.zeros((128, NSB * 32, 4), np.float32)
        mdiag[p, :, gl_p] = val
        mb = sum_tab[mem.reshape(NT, 128)]  # [NT, 128, D] f32
        memb_nat = np.ascontiguousarray(
            (mb * MT_SCALE).astype(f8).transpose(1, 0, 2)
        ).reshape(128, NT * D)
        membt = np.ascontiguousarray(
            (mb.reshape(NT * 128, D) * MT_SCALE).T.astype(f8)
        )
        m = dict(
            consts,
            memb=memb_nat,
            membt=membt,
            mdiag=np.ascontiguousarray(mdiag.reshape(128, NT * 4)).astype(bf),
            gtabt=np.ascontiguousarray(gtab_full[sl].T * MT_SCALE),
            ht=np.ascontiguousarray(Hg[sl].T[perm]).astype(bf),
        )
        in_maps.append(m)
    return in_maps


def kernel(**inputs):
    if "nc" not in _CACHE:
        _CACHE["nc"] = _build()
    nc = _CACHE["nc"]
    in_maps = _prep_inputs(**inputs)
    res = run_bass_kernel_spmd(nc, in_maps, core_ids=list(range(NC)))
    out = np.concatenate(
        [np.ascontiguousarray(res.results[c]["outt"].T) for c in range(NC)], axis=0
    )
    return out.astype(np.float32)


if __name__ == "__main__":
    import reference
    inp = {k: np.asarray(v) for k, v in reference.setup_inputs().items()}
    exp = np.asarray(reference.reference(**inp))
    got = kernel(**inp)
    err = np.abs(got - exp).max() / (np.abs(exp).max() + 1e-30)
    rel = np.linalg.norm(got - exp) / (np.linalg.norm(exp) + 1e-30)
    print(f"absmax-rel: {err:.3e}  fro-rel: {rel:.3e}")


# revision 22
# speedup vs baseline: 1.2316x; 1.2316x over previous
"""Trainium2 Bass kernel for nn_HHGR (gnn_message_passing).

Strategy (8 NeuronCores, groups sharded 1024/core):
  host prep: sum_tab = user_table + user_embedding; two member slabs per
  core: memb (natural [128, NT*128] bf16 rows = member rows, used as the
  g_att stationary) and membt (transposed [D, R] fp8e4 scaled x16, used
  as the moving operand of the hidden-layer matmul -> no PE transposes);
  H^T slab bf16 with k-rows permuted to the AllGather-half layout;
  gtab^T; mask block-diag in (slice, chunk) column order so batched
  logits land contiguously; replicated attention weights.

  device per core (1024 groups = 8 superblocks x 128 groups):
  * hid^T = aw1^T @ membt per 512-row chunk, 3 chunks stacked per PSUM
    bank (partition offsets 0/32/64); batched Relu+bias per stack;
    logits batched: lhsT = stacked hidT 128-col slice, rhs = aw2sel
    [128, 3] -> 512/384 member rows per matmul (96 matmuls/core vs 256).
  * softmax: one Exp+bias; pm = p*mask; dent = ones^T @ pm (direct
    transposed denominators, no PE transpose); denbc via 4 K=1 ind4
    matmuls; maskp = pm * recip.
  * g_att^T accumulated per 128-row tile (lhsT = memb tile, rhs = maskp
    4 cols); X^T = g_att^T + gtab^T; V = X @ hw1 (natural rows).
  * software pipelining: exp/pm of sb issue before hid/logits of sb+1,
    then the softmax-dependent PE work of sb — the DVE chain hides
    under the next superblock's matmuls.
  * V AllGathered in 2 halves (after superblocks 3/7); W in 2 halves
    produced by column-half-major stage 1. All collective triggers and
    slab-prefetch DMA triggers live on the GpSimd queue; vd stores on
    the vector queue right after the V copy.
  * stage 1 (Y^T = V^T H^T, K=8192) column-half-major, k-ordered so the
    second V half is consumed last (ch0) / first (ch1); N=512 matmuls.
  * stage 2 (out^T = W^T H^T) k-ordered by W half arrival; out^T
    [128, 1024] f32 written once and transposed on the host.
"""
import sys
sys.path.insert(0, "/opt/trn_rl_repo")

import numpy as np
import ml_dtypes

import concourse.bass as bass  # noqa: F401
import concourse.bacc as bacc
import concourse.mybir as mybir
import concourse.tile as tile
from concourse.bass_utils import run_bass_kernel_spmd

F32 = mybir.dt.float32
BF16 = mybir.dt.bfloat16
FP8 = mybir.dt.float8e4
AF = mybir.ActivationFunctionType

G, M, D, U = 8192, 32, 128, 200000
H_ATT = 16
NC = 8
GPC = G // NC          # 1024 groups per core
R = GPC * M            # 32768 member rows per core
NT = R // 128          # 256 tiles of 128 rows
NSB = GPC // 128       # 8 superblocks of 128 groups (32 tiles each)
KT = G // 128          # 64 k-tiles for the big matmuls
MT_SCALE = 16.0        # membt fp8 pre-scale (folded into aw1)

_CACHE = {}


def _build():
    nc = bacc.Bacc("TRN2", target_bir_lowering=False, debug=False)

    memb = nc.dram_tensor("memb", [128, NT * D], FP8, kind="ExternalInput")
    membt = nc.dram_tensor("membt", [128, NT * D], FP8, kind="ExternalInput")
    mdiag = nc.dram_tensor("mdiag", [128, NT * 4], BF16, kind="ExternalInput")
    gtabt = nc.dram_tensor("gtabt", [D, GPC], F32, kind="ExternalInput")
    ht = nc.dram_tensor("ht", [G, GPC], BF16, kind="ExternalInput")
    aw1 = nc.dram_tensor("aw1", [D, H_ATT], BF16, kind="ExternalInput")
    aw2sel = nc.dram_tensor("aw2sel", [128, 3], BF16, kind="ExternalInput")
    ab1s = nc.dram_tensor("ab1s", [128, 1], F32, kind="ExternalInput")
    ab2s = nc.dram_tensor("ab2s", [128, 1], F32, kind="ExternalInput")
    hw1 = nc.dram_tensor("hw1", [D, D], BF16, kind="ExternalInput")
    hw2 = nc.dram_tensor("hw2", [D, D], BF16, kind="ExternalInput")
    ind4 = nc.dram_tensor("ind4", [1, 512], F32, kind="ExternalInput")
    ones_col = nc.dram_tensor("ones_col", [128, 1], BF16, kind="ExternalInput")

    outt = nc.dram_tensor("outt", [D, GPC], F32, kind="ExternalOutput")

    # collective staging: per-half V/W rows, tiled [128, 4*128] so the
    # post-AllGather reload is 1KB-contiguous per partition.
    vd = nc.dram_tensor("vd", [128, GPC], BF16)
    vag = nc.dram_tensor("vag", [NC * 128, GPC], BF16, addr_space="Shared")
    wd = [nc.dram_tensor(f"wd{h}", [128, 512], BF16) for h in range(2)]
    wag = [
        nc.dram_tensor(f"wag{h}", [NC * 128, 512], BF16, addr_space="Shared")
        for h in range(2)
    ]

    def allgather(src, dst):
        nc.gpsimd.collective_compute(
            "AllGather",
            mybir.AluOpType.bypass,
            replica_groups=[list(range(NC))],
            ins=[src[:]],
            outs=[dst[:]],
        )

    with tile.TileContext(nc) as tc:
        with (
            tc.tile_pool(name="const", bufs=1) as cpool,
            tc.tile_pool(name="htpool", bufs=1) as htpool,
            tc.tile_pool(name="gath", bufs=2) as gpool,
            tc.tile_pool(name="gatht", bufs=2) as tpool,
            tc.tile_pool(name="kagg", bufs=1) as kpool,
            tc.tile_pool(name="work", bufs=2) as wpool,
            tc.tile_pool(name="big", bufs=1) as xpool,
            tc.tile_pool(name="ps_h", bufs=2, space="PSUM") as psh,
            tc.tile_pool(name="ps_m", bufs=2, space="PSUM") as psm,
            tc.tile_pool(name="ps_w", bufs=2, space="PSUM") as psw,
            tc.tile_pool(name="ps_y", bufs=1, space="PSUM") as psy,
        ):
            # ---- constants ----
            aw1_sb = cpool.tile([D, H_ATT], BF16, tag="aw1")
            nc.sync.dma_start(aw1_sb[:], aw1[:])
            aw2sel_sb = cpool.tile([128, 3], BF16, tag="aw2sel")
            nc.sync.dma_start(aw2sel_sb[:], aw2sel[:])
            ab1s_sb = cpool.tile([128, 1], F32, tag="ab1s")
            nc.sync.dma_start(ab1s_sb[:], ab1s[:])
            ab2s_sb = cpool.tile([128, 1], F32, tag="ab2s")
            nc.sync.dma_start(ab2s_sb[:], ab2s[:])
            hw1_sb = cpool.tile([D, D], BF16, tag="hw1")
            nc.sync.dma_start(hw1_sb[:], hw1[:])
            hw2_sb = cpool.tile([D, D], BF16, tag="hw2")
            nc.sync.dma_start(hw2_sb[:], hw2[:])
            ind4_sb = cpool.tile([1, 512], F32, tag="ind4")
            nc.sync.dma_start(ind4_sb[:], ind4[:])
            ones_sb = cpool.tile([128, 1], BF16, tag="ones")
            nc.sync.dma_start(ones_sb[:], ones_col[:])
            mdiag_sb = cpool.tile([128, NT * 4], BF16, tag="mdiag")
            nc.sync.dma_start(mdiag_sb[:], mdiag[:])
            gtabt_sb = cpool.tile([D, GPC], F32, tag="gtabt")
            nc.sync.dma_start(gtabt_sb[:], gtabt[:])

            # engine warmups: first ACT/DVE ops pay a ucode library
            # load; do it on tiny data at t0 instead of mid-pipeline
            wu_sb = cpool.tile([128, 2], F32, tag="wusb")
            nc.scalar.activation(wu_sb[:, 0:1], ab2s_sb[:], AF.Exp)
            nc.vector.reciprocal(wu_sb[:, 1:2], wu_sb[:, 0:1])

            # ---- H^T resident [128, KT*GPC] bf16 (16MB) ----
            # NOT loaded up-front: the 64 k-tile loads are issued from the
            # scalar queue inside the attention loop (paced by the exp data
            # dependency), so the member slabs get full HBM bandwidth first
            # and the attention runs PE-bound instead of DMA-starved.
            ht_sb = htpool.tile([128, KT * GPC], BF16, tag="ht")

            def load_ht(k):
                nc.gpsimd.dma_start(
                    ht_sb[:, k * GPC : (k + 1) * GPC],
                    ht[k * 128 : (k + 1) * 128, :],
                )

            # ---- member slab prefetch (gpsimd trigger queue) ----
            def load_gbig(sb):
                t = gpool.tile([128, 32 * 128], FP8, tag="gbig", name=f"gbig{sb}")
                nc.gpsimd.dma_start(
                    t[:], memb[:, sb * 32 * 128 : (sb + 1) * 32 * 128]
                )
                return t

            def load_gbt(sb):
                t = tpool.tile([128, 32 * 128], FP8, tag="gbt", name=f"gbt{sb}")
                nc.gpsimd.dma_start(
                    t[:], membt[:, sb * 32 * 128 : (sb + 1) * 32 * 128]
                )
                return t

            gbt = [None] * NSB
            gbig = [None] * NSB
            gbt[0] = load_gbt(0)
            gbig[0] = load_gbig(0)
            if NSB > 1:
                gbt[1] = load_gbt(1)
                gbig[1] = load_gbig(1)

            miscs = [None] * NSB

            def attn_A(sb):
                """hid + logits for superblock sb -> logit area of misc."""
                misc = psm.tile([128, 512], F32, tag="misc", name=f"misc{sb}")
                miscs[sb] = misc
                logit_ps = misc[:, 0:32]
                hid_list = []
                # stacks: q=0 chunks 0-2, q=1 chunks 3-5, q=2 chunks 6-7
                for q in range(3):
                    nch = 3 if q < 2 else 2
                    hid_ps = psh.tile([128, 512], F32, tag="hid",
                                      name=f"hid{sb}_{q}")
                    for j in range(nch):
                        c = 3 * q + j
                        nc.tensor.matmul(
                            hid_ps[32 * j : 32 * j + H_ATT, :],
                            aw1_sb[:],
                            gbt[sb][:, c * 512 : (c + 1) * 512],
                            start=True,
                            stop=True,
                        )
                    hid_list.append((hid_ps, nch))
                # relu (ACT) + batched logits; interleave so ACT of stack q
                # overlaps PE of stack q+1's hid above / logits below
                for q, (hid_ps, nch) in enumerate(hid_list):
                    hidT = wpool.tile([128, 512], BF16, tag="hidT",
                                      name=f"hidT{sb}_{q}")
                    nc.scalar.activation(
                        hidT[:], hid_ps[:], AF.Relu, bias=ab1s_sb[:, :1]
                    )
                    for s in range(4):
                        # logit col = s*8 + c for chunk c = 3q+j
                        nc.tensor.matmul(
                            logit_ps[:, s * 8 + 3 * q : s * 8 + 3 * q + nch],
                            hidT[:, s * 128 : (s + 1) * 128],
                            aw2sel_sb[:, 0:nch],
                            start=True,
                            stop=True,
                        )

            def attn_exp(sb):
                """early ACT/DVE for B(sb): exp + mask-multiply."""
                misc = miscs[sb]
                p_sb = wpool.tile([128, 32], BF16, tag="p", name=f"p{sb}")
                nc.scalar.activation(
                    p_sb[:], misc[:, 0:32], AF.Exp, bias=ab2s_sb[:, :1]
                )
                pm_sb = wpool.tile([128, 128], BF16, tag="pm", name=f"pm{sb}")
                nc.vector.tensor_tensor(
                    pm_sb[:].rearrange("p (t l) -> p t l", l=4),
                    p_sb[:].rearrange("p (t o) -> p t o", o=1).to_broadcast(
                        [128, 32, 4]
                    ),
                    mdiag_sb[:, sb * 128 : (sb + 1) * 128].rearrange(
                        "p (t l) -> p t l", l=4
                    ),
                    mybir.AluOpType.mult,
                )
                return pm_sb

            def attn_B(sb, pm_sb):
                """softmax denominators + g_att + V for superblock sb."""
                misc = miscs[sb]
                dent_ps = misc[0:1, 32:160]
                denbc_ps = misc[:, 160:192]
                gatt_ps = misc[:, 192:320]
                v_ps = misc[:, 320:448]

                # transposed denominators: dent[0, col] = sum_rows pm[row, col]
                nc.tensor.matmul(dent_ps, ones_sb[:], pm_sb[:],
                                 start=True, stop=True)
                dent_sb = wpool.tile([1, 128], F32, tag="dent", name=f"dent{sb}")
                nc.vector.tensor_copy(dent_sb[:], dent_ps)
                # den_bc[r, j] = den[(j, r//32)] via 4 K=1 matmuls
                dent_re = dent_sb[:].rearrange("p (t l) -> p t l", l=4)
                for gl in range(4):
                    nc.tensor.matmul(
                        denbc_ps,
                        ind4_sb[0:1, gl * 128 : (gl + 1) * 128],
                        dent_re[:, :, gl : gl + 1],
                        start=(gl == 0),
                        stop=(gl == 3),
                    )
                recip_sb = wpool.tile([128, 32], F32, tag="recip",
                                      name=f"recip{sb}")
                nc.vector.reciprocal(recip_sb[:], denbc_ps)
                maskp_sb = wpool.tile([128, 128], BF16, tag="maskp",
                                      name=f"maskp{sb}")
                nc.vector.tensor_tensor(
                    maskp_sb[:].rearrange("p (t l) -> p t l", l=4),
                    recip_sb[:].rearrange("p (t o) -> p t o", o=1).to_broadcast(
                        [128, 32, 4]
                    ),
                    pm_sb[:].rearrange("p (t l) -> p t l", l=4),
                    mybir.AluOpType.mult,
                )
                # g_att^T accumulation: [128 d, 128 groups]; tile t's
                # weights live at logit col j = (t%4)*8 + t//4
                for t in range(32):
                    jt = (t % 4) * 8 + t // 4
                    nc.tensor.matmul(
                        gatt_ps[:, 4 * t : 4 * t + 4],
                        gbig[sb][:, t * 128 : (t + 1) * 128],
                        maskp_sb[:, 4 * jt : 4 * jt + 4],
                        start=True,
                        stop=True,
                    )
                xt_sb = wpool.tile([128, 128], BF16, tag="xt", name=f"xt{sb}")
                nc.vector.tensor_tensor(
                    xt_sb[:],
                    gatt_ps,
                    gtabt_sb[:, sb * 128 : (sb + 1) * 128],
                    mybir.AluOpType.add,
                )
                nc.tensor.matmul(v_ps, xt_sb[:], hw1_sb[:], start=True, stop=True)
                v_sb = wpool.tile([128, 128], BF16, tag="v", name=f"v{sb}")
                nc.vector.tensor_copy(v_sb[:], v_ps)
                nc.sync.dma_start(
                    vd[:, sb * 128 : (sb + 1) * 128], v_sb[:]
                )

            # ---- attention main loop (software pipelined) ----
            ht_per_sb = [0, 2, 4, 6, 8, 9, 10, 0]
            ht_next = 0
            attn_A(0)
            for sb in range(NSB):
                if sb + 2 < NSB:
                    gbt[sb + 2] = load_gbt(sb + 2)
                    gbig[sb + 2] = load_gbig(sb + 2)
                for _ in range(ht_per_sb[sb]):
                    load_ht(ht_next)
                    ht_next += 1
                pm_sb = attn_exp(sb)
                if sb + 1 < NSB:
                    attn_A(sb + 1)
                attn_B(sb, pm_sb)
                if sb == NSB - 1:
                    allgather(vd, vag)
            # ht tail: issued after the AG doorbell so the gpsimd queue
            # reaches the doorbell with no slot barriers ahead of it
            while ht_next < KT:
                load_ht(ht_next)
                ht_next += 1
            vag_sb = kpool.tile([128, NC * GPC], BF16, tag="kqv")
            for q in range(4):
                c0, c1 = 2 * q, 2 * q + 2
                nc.sync.dma_start(
                    vag_sb[:, c0 * GPC : c1 * GPC].rearrange(
                        "p (c f) -> p c f", f=GPC
                    ),
                    vag[c0 * 128 : c1 * 128, :].rearrange(
                        "(c p) f -> p c f", p=128
                    ),
                )

            # ---- stage 1: Y^T = V^T H^T ----
            # blocks: (vh0,ch0), (vh0,ch1), (vh1,ch0) -> y0 done at the 3/4
            # point -> W half 0 + AllGather launch early, then (vh1,ch1).
            ht_all = xpool.tile([128, GPC], BF16, tag="hT")
            wag_sb = [None, None]
            y_tiles = [
                psy.tile([128, 512], F32, tag="y0", name="y_ps0"),
                psy.tile([128, 512], F32, tag="y1", name="y_ps1"),
            ]

            def finish_w_half(ch):
                nc.scalar.activation(
                    ht_all[:, ch * 512 : (ch + 1) * 512], y_tiles[ch][:], AF.Relu
                )
                w_half = wpool.tile([128, 512], BF16, tag="w",
                                    name=f"w{ch}")
                for gb in range(4):
                    g0 = ch * 4 + gb
                    w_ps = psw.tile([128, 128], F32, tag="wps",
                                    name=f"wps{ch}_{gb}")
                    nc.tensor.matmul(
                        w_ps[:],
                        ht_all[:, g0 * 128 : (g0 + 1) * 128],
                        hw2_sb[:],
                        start=True,
                        stop=True,
                    )
                    nc.vector.tensor_copy(
                        w_half[:, gb * 128 : (gb + 1) * 128], w_ps[:]
                    )
                nc.sync.dma_start(wd[ch][:], w_half[:])
                allgather(wd[ch], wag[ch])

            for ch in range(2):
                for kt in range(KT):
                    nc.tensor.matmul(
                        y_tiles[ch][:],
                        vag_sb[:, kt * 128 : (kt + 1) * 128],
                        ht_sb[:, kt * GPC + ch * 512 : kt * GPC + ch * 512 + 512],
                        start=(kt == 0),
                        stop=(kt == KT - 1),
                    )
                finish_w_half(ch)
            for ch in range(2):
                wag_sb[ch] = kpool.tile(
                    [128, NC * 512], BF16, tag=f"kqw{ch}", name=f"wag_sb{ch}"
                )
                hw = NC // 2
                nc.sync.dma_start(
                    wag_sb[ch][:, 0 : hw * 512].rearrange("p (c f) -> p c f", f=512),
                    wag[ch][0 : hw * 128, :].rearrange("(c p) f -> p c f", p=128),
                )
                nc.sync.dma_start(
                    wag_sb[ch][:, hw * 512 :].rearrange("p (c f) -> p c f", f=512),
                    wag[ch][hw * 128 :, :].rearrange("(c p) f -> p c f", p=128),
                )

            # ---- stage 2: out^T = W^T H^T, k-ordered by W half ----
            o_ps = [
                psy.tile([128, 512], F32, tag="y0", name="o_ps0"),
                psy.tile([128, 512], F32, tag="y1", name="o_ps1"),
            ]
            for c2 in range(2):
                first = True
                for wh in range(2):
                    for c in range(NC):
                        for gb in range(4):
                            kidx = c * 8 + wh * 4 + gb
                            last = wh == 1 and c == NC - 1 and gb == 3
                            nc.tensor.matmul(
                                o_ps[c2][:],
                                wag_sb[wh][:, (c * 4 + gb) * 128 : (c * 4 + gb + 1) * 128],
                                ht_sb[:, kidx * GPC + c2 * 512 : kidx * GPC + c2 * 512 + 512],
                                start=first,
                                stop=last,
                            )
                            first = False
                # copy/store of this half overlaps the next pass's matmuls
                ot_sb = xpool.tile([128, 512], F32, tag=f"ot{c2}")
                nc.vector.tensor_copy(ot_sb[:], o_ps[c2][:])
                nc.sync.dma_start(outt[:, c2 * 512 : (c2 + 1) * 512], ot_sb[:])

    nc.compile()
    return nc


def _prep_inputs(group_inputs, members, member_mask, user_embedding, H_gl,
                 user_table, group_table, aw1, ab1, aw2, ab2, hw1, hw2):
    bf = ml_dtypes.bfloat16
    f8 = ml_dtypes.float8_e4m3
    sum_tab = (
        np.asarray(user_table, np.float32) + np.asarray(user_embedding, np.float32)
    )
    gi = np.asarray(group_inputs, np.int64)
    gtab_full = np.asarray(group_table, np.float32)[gi]
    Hg = np.asarray(H_gl, np.float32)

    aw2v = np.asarray(aw2, np.float32).reshape(-1)
    ab1v = np.asarray(ab1, np.float32).reshape(-1)
    aw2sel = np.zeros((128, 3), np.float32)
    ab1s = np.zeros((128, 1), np.float32)
    for j in range(4):
        ab1s[32 * j : 32 * j + H_ATT, 0] = ab1v
        if j < 3:
            aw2sel[32 * j : 32 * j + H_ATT, j] = aw2v
    ab2s = np.full((128, 1), np.asarray(ab2, np.float32).reshape(-1)[0], np.float32)
    ind4 = np.zeros((1, 512), np.float32)
    for gl in range(4):
        ind4[0, gl * 128 + 32 * gl : gl * 128 + 32 * (gl + 1)] = 1.0

    # single-V-AllGather layout: k-tile kt = c*8 + j holds global groups
    # c*GPC + j*128 + p, i.e. the identity ordering
    perm = np.arange(G)

    consts = dict(
        aw1=(np.asarray(aw1, np.float32) / MT_SCALE).astype(bf),
        aw2sel=aw2sel.astype(bf),
        ab1s=ab1s,
        ab2s=ab2s,
        hw1=(np.asarray(hw1, np.float32) / MT_SCALE).astype(bf),
        hw2=np.asarray(hw2, np.float32).astype(bf),
        ind4=ind4,
        ones_col=np.ones((128, 1), np.float32).astype(bf),
    )

    p = np.arange(128)
    gl_p = p // 32
    m_p = p % 32
    # logit col j covers superblock-local tile t(j) = 4*(j%8) + j//8
    t_of_j = 4 * (np.arange(32) % 8) + np.arange(32) // 8
    cols = np.arange(NSB * 32)                       # (sb, j) column index
    grp_of_col = (cols // 32) * 128 + 4 * t_of_j[cols % 32]  # local group base
    in_maps = []
    for c in range(NC):
        sl = slice(c * GPC, (c + 1) * GPC)
        mem = np.asarray(members, np.int64)[sl].astype(np.int32).reshape(-1)
        mask01 = (np.asarray(member_mask, np.float32)[sl] > 0).astype(np.float32)
        # mdiag[p, (sb, j, l)] = mask of (group base+l, member p%32) if p//32==l
        val = mask01[(grp_of_col[None, :] + gl_p[:, None]), m_p[:, None]]
        mdiag = np.zeros((128, NSB * 32, 4), np.float32)
        mdiag[p, :, gl_p] = val
        mb = sum_tab[mem.reshape(NT, 128)]  # [NT, 128, D] f32
        memb_nat = np.ascontiguousarray(
            (mb * MT_SCALE).astype(f8).transpose(1, 0, 2)
        ).reshape(128, NT * D)
        membt = np.ascontiguousarray(
            (mb.reshape(NT * 128, D) * MT_SCALE).T.astype(f8)
        )
        m = dict(
            consts,
            memb=memb_nat,
            membt=membt,
            mdiag=np.ascontiguousarray(mdiag.reshape(128, NT * 4)).astype(bf),
            gtabt=np.ascontiguousarray(gtab_full[sl].T * MT_SCALE),
            ht=np.ascontiguousarray(Hg[sl].T[perm]).astype(bf),
        )
        in_maps.append(m)
    return in_maps


def kernel(**inputs):
    if "nc" not in _CACHE:
        _CACHE["nc"] = _build()
    nc = _CACHE["nc"]
    in_maps = _prep_inputs(**inputs)
    res = run_bass_kernel_spmd(nc, in_maps, core_ids=list(range(NC)))
    out = np.concatenate(
        [np.ascontiguousarray(res.results[c]["outt"].T) for c in range(NC)], axis=0
    )
    return out.astype(np.float32)


if __name__ == "__main__":
    import reference
    inp = {k: np.asarray(v) for k, v in reference.setup_inputs().items()}
    exp = np.asarray(reference.reference(**inp))
    got = kernel(**inp)
    err = np.abs(got - exp).max() / (np.abs(exp).max() + 1e-30)
    rel = np.linalg.norm(got - exp) / (np.linalg.norm(exp) + 1e-30)
    print(f"absmax-rel: {err:.3e}  fro-rel: {rel:.3e}")
